# revision 8
# baseline (speedup 1.0000x reference)
"""Trainium2 Bass kernel: LSTM (B=4096, S=512, I=8, H=64) + FC(4) + softmax.

Data-parallel over 8 NeuronCores: each core owns 512 batch rows and runs the
full 512-step recurrence on them. Layout on device is feature-major
([features, batch]) so h feeds the next step's matmul with no transposes.

Device math per step (per batch half of 256):
  psum[128,512] = Wcat.T @ [h~; x_t; 1]   (2 matmuls, K=73, fp32 accum)
      cols 0:256  -> gates [i; f],  cols 256:512 -> gates [2g; o]
  s = sigmoid(psum)                        (1 ACT op; tanh comes from sigma)
  v~ = (p - 0.5) * s_i                     (GPSIMD fused op; p = sigma(2g))
  w  = s_f * C                             (DVE;  C = 2c)
  C  = 4*v~ + w                            (DVE fused)
  q  = sigmoid(C)                          (ACT; = sigma(2c))
  h~ = (q - 0.5) * s_o                     (DVE fused; h~ = h/2)
All affine fixes are folded into host-prepped weights: W_hh and W_fc are
pre-doubled (h~ convention), g-gate columns pre-doubled (sigma trick), and
the bias rides in the matmul as a ones row of the rhs.
"""

import numpy as np
import ml_dtypes
from contextlib import ExitStack

import concourse.bass as bass
import concourse.tile as tile
from concourse import bacc
from concourse import mybir
from concourse.bass_utils import run_bass_kernel_spmd

B, S, I, H, O = 4096, 512, 8, 64, 4
NCORES = 8
BLOC = B // NCORES          # 512 batch rows per core
HALF = BLOC // 2            # 256 (batch split for pipelining)
TPB = 16                    # timesteps per DMA block
KDIM = H + I + 1            # 73 = h rows + x rows + ones row
G4 = 4 * H                  # 256

F32 = mybir.dt.float32
BF16 = mybir.dt.bfloat16
AF = mybir.ActivationFunctionType
ALU = mybir.AluOpType
BF16NP = ml_dtypes.bfloat16

_CACHE = {}
LAST = {"exec_time_ns": None}


def build_nc(steps=S):
    nblk = steps // TPB
    assert steps % TPB == 0
    nc = bacc.Bacc()
    xT = nc.declare_dram_parameter("xT", [steps, I + 1, BLOC], BF16, isOutput=False)
    wcat = nc.declare_dram_parameter("wcat", [KDIM, G4], BF16, isOutput=False)
    wfc = nc.declare_dram_parameter("wfc", [H + 1, O], BF16, isOutput=False)
    out = nc.declare_dram_parameter("out", [BLOC, O], F32, isOutput=True)

    with ExitStack() as ctx:
        tc = ctx.enter_context(tile.TileContext(nc))
        singles = ctx.enter_context(tc.tile_pool(name="singles", bufs=1))
        rpool = ctx.enter_context(tc.tile_pool(name="rpool", bufs=2))
        psums = ctx.enter_context(tc.tile_pool(name="psums", bufs=3, space="PSUM"))
        fcpsum = ctx.enter_context(tc.tile_pool(name="fcpsum", bufs=2, space="PSUM"))
        sact = ctx.enter_context(tc.tile_pool(name="sact", bufs=4))
        small = ctx.enter_context(tc.tile_pool(name="small", bufs=4))

        w_sb = singles.tile([KDIM, G4], BF16)
        nc.sync.dma_start(out=w_sb[:, :], in_=wcat[:, :])
        wfc_sb = singles.tile([H + 1, O], BF16)
        nc.sync.dma_start(out=wfc_sb[:, :], in_=wfc[:, :])

        C_half = [singles.tile([H, HALF], BF16, tag=f"C{k}", name=f"C{k}") for k in range(2)]
        for t_ in C_half:
            nc.vector.memset(t_[:, :], 0.0)

        hfin = singles.tile([H + 1, BLOC], BF16)
        nc.vector.memset(hfin[H:H + 1, :], 1.0)

        def new_rtile(blk):
            R = rpool.tile([KDIM, TPB * BLOC], BF16, tag="R")
            nc.sync.dma_start(
                out=R[H:KDIM, :].rearrange("p (t c) -> p t c", t=TPB),
                in_=xT[blk * TPB:(blk + 1) * TPB, :, :].rearrange("t i c -> i t c"),
            )
            return R

        R_cur = new_rtile(0)
        nc.vector.memset(R_cur[0:H, 0:BLOC], 0.0)  # h_0 = 0 in slot 0

        for blk in range(nblk):
            R_next = new_rtile(blk + 1) if blk + 1 < nblk else None
            for st in range(TPB):
                last_step = (blk == nblk - 1) and (st == TPB - 1)
                for half in range(2):
                    c0 = st * BLOC + half * HALF
                    rhs = R_cur[:, c0:c0 + HALF]
                    # All 4 gates at partitions 0:64 (M=64 matmuls) so every
                    # downstream elementwise pair is base-partition aligned.
                    ps = psums.tile([H, 4 * HALF], F32, tag="ps")
                    for g in range(4):
                        nc.tensor.matmul(ps[:, g * HALF:(g + 1) * HALF],
                                         w_sb[:, g * H:(g + 1) * H], rhs,
                                         start=True, stop=True)
                    s_all = sact.tile([H, 4 * HALF], BF16, tag="sall")
                    nc.scalar.activation(s_all[:, :], ps[:, :], AF.Sigmoid)
                    s_i = s_all[:, 0:HALF]
                    s_f = s_all[:, HALF:2 * HALF]
                    p_g = s_all[:, 2 * HALF:3 * HALF]
                    s_o = s_all[:, 3 * HALF:4 * HALF]

                    vt = small.tile([H, HALF], BF16, tag="vt")
                    nc.vector.scalar_tensor_tensor(vt[:, :], p_g, -0.5, s_i,
                                                   ALU.add, ALU.mult)
                    wt = small.tile([H, HALF], BF16, tag="wt")
                    nc.gpsimd.tensor_mul(wt[:, :], s_f, C_half[half][:, :])
                    nc.vector.scalar_tensor_tensor(C_half[half][:, :], vt[:, :], 4.0,
                                                   wt[:, :], ALU.mult, ALU.add)
                    qt = small.tile([H, HALF], BF16, tag="qt")
                    nc.scalar.activation(qt[:, :], C_half[half][:, :], AF.Sigmoid)

                    if last_step:
                        h_dst = hfin[0:H, half * HALF:half * HALF + HALF]
                    elif st + 1 < TPB:
                        d0 = (st + 1) * BLOC + half * HALF
                        h_dst = R_cur[0:H, d0:d0 + HALF]
                    else:
                        d0 = half * HALF
                        h_dst = R_next[0:H, d0:d0 + HALF]
                    nc.vector.scalar_tensor_tensor(h_dst, qt[:, :], -0.5, s_o,
                                                   ALU.add, ALU.mult)
            R_cur = R_next

        # FC + softmax over the 4 logits (free dim), batch on partitions.
        for m in range(BLOC // 128):
            psf = fcpsum.tile([128, O], F32, tag="psf")
            nc.tensor.matmul(psf[:, :], hfin[:, m * 128:(m + 1) * 128],
                             wfc_sb[:, :], start=True, stop=True)
            mx = small.tile([128, 1], F32, tag="mx")
            nc.vector.tensor_reduce(mx[:, :], psf[:, :], mybir.AxisListType.X,
                                    ALU.max)
            mxn = small.tile([128, 1], F32, tag="mxn")
            nc.vector.tensor_scalar_mul(mxn[:, :], mx[:, :], -1.0)
            e = small.tile([128, O], F32, tag="e")
            nc.scalar.activation(e[:, :], psf[:, :], AF.Exp, bias=mxn[:, :])
            sm = small.tile([128, 1], F32, tag="sm")
            nc.vector.tensor_reduce(sm[:, :], e[:, :], mybir.AxisListType.X,
                                    ALU.add)
            rc = small.tile([128, 1], F32, tag="rc")
            nc.vector.reciprocal(rc[:, :], sm[:, :])
            og = small.tile([128, O], F32, tag="og")
            nc.vector.tensor_scalar_mul(og[:, :], e[:, :], rc[:, :])
            nc.sync.dma_start(out=out[m * 128:(m + 1) * 128, :], in_=og[:, :])
    nc.compile()
    return nc


def prep_weights(W_ih, W_hh, b_ih, b_hh, W_fc, b_fc):
    b = (b_ih + b_hh).astype(np.float32)
    W_cat = np.concatenate(
        [2.0 * W_hh.T, W_ih.T, b[None, :]], axis=0).astype(np.float32)
    W_cat[:, 128:192] *= 2.0
    rhs_fc = np.concatenate(
        [2.0 * W_fc.T, b_fc[None, :]], axis=0).astype(np.float32)
    return W_cat.astype(BF16NP), rhs_fc.astype(BF16NP)


def kernel(x, W_ih, W_hh, b_ih, b_hh, W_fc, b_fc, steps=S, trace=False):
    x = np.asarray(x, dtype=np.float32)
    wcat, wfc = prep_weights(np.asarray(W_ih, np.float32), np.asarray(W_hh, np.float32),
                             np.asarray(b_ih, np.float32), np.asarray(b_hh, np.float32),
                             np.asarray(W_fc, np.float32), np.asarray(b_fc, np.float32))
    if steps not in _CACHE:
        _CACHE[steps] = build_nc(steps)
    nc = _CACHE[steps]

    in_maps = []
    for s in range(NCORES):
        xs = x[s * BLOC:(s + 1) * BLOC, :steps, :]          # [BLOC, steps, I]
        xT = np.ascontiguousarray(xs.transpose(1, 2, 0))    # [steps, I, BLOC]
        xTo = np.concatenate(
            [xT, np.ones((steps, 1, BLOC), np.float32)], axis=1).astype(BF16NP)
        in_maps.append({"xT": xTo, "wcat": wcat, "wfc": wfc})

    LAST["in_maps"] = in_maps
    res = run_bass_kernel_spmd(nc, in_maps, list(range(NCORES)), trace=trace)
    LAST["exec_time_ns"] = res.exec_time_ns
    outs = [np.asarray(res.results[i]["out"], np.float32) for i in range(NCORES)]
    return np.concatenate(outs, axis=0)
